# revision 2
# baseline (speedup 1.0000x reference)
"""CrossAttentionBlock kernel for 8 trn2 NeuronCores — v2.

Sharding: core c = b*4 + hg handles batch b (of 2) and head-group hg
(4 of the 16 heads). Host sums the 4 partial out-projections per batch.

v2 redesign vs baseline (ACT-exp bound, 218us):
  - attn@V flipped: pt (exp(S), bf16) is the STATIONARY, v the moving
    operand -> out[n-sub, 65] in 65-cycle full-rate streams (~2x less
    PE than the old [65, 512] form). Softmax denominators land as a
    per-partition column (v ones column), so normalization is a DVE
    tensor_scalar; the old PE broadcast matmuls + norm copies die.
  - normalized [n, d] tiles go back to [d, n] via a small fp32 PE
    transpose (identity stationary) into a scratch-PSUM view.
  - rstd = Exp(-0.5*Ln(var+eps)) on ACT: same table set as the
    attention exp => ONE act-table load for the whole kernel, which
    lets Q-proj chunks pipeline INTO attention without table thrash.
  - Q-proj chunk ch+1 is emitted as small quanta popped in attention
    (ch)'s m-slots (PE has slack under the ACT-bound cadence).
  - PSUM = 8 banks exactly: st pair ring 2x2, scratch 2 (tag rings
    "qps" long-lived / "short" 1-slot: proj-mm, pj, stat halves,
    transpose views), ot 2 (otA/otB; double as V-proj psum in the KV
    phase). OT accumulates 4 n-sub regions per bank: only the first
    matmul touching a bank each round uses start=True (has_written
    clear is bank-wide).
"""

import sys

import numpy as np

if "/opt/trn_rl_repo" not in sys.path:
    sys.path.insert(0, "/opt/trn_rl_repo")

import concourse.bacc as bacc
import concourse.tile as tile
from concourse import mybir
from concourse.bass_utils import run_bass_kernel_spmd

F32 = mybir.dt.float32
F32R = mybir.dt.float32r
BF16 = mybir.dt.bfloat16
AF = mybir.ActivationFunctionType
ALU = mybir.AluOpType

C = 1024          # model dim
NT = 2048         # sequence length (N == M)
HD = 64           # head dim
NHL = 4           # heads per core
DL = NHL * HD     # 256 local channel width
P = 128           # partitions
CH = 512          # n-chunk
NCH = NT // CH    # 4 chunks
MTILES = NT // P  # 16 m-tiles
SCALE = HD ** -0.5
LN_EPS = 1e-5
VW = HD + 1       # 65: v block per head: [v(64), ones column]
# Fast rsqrt seed on ACT (exp table set, no sqrt-table thrash):
# y0 = Exp(RSTD_SCALE * bitcast_i32(var+eps) + RSTD_BIAS) ~ (var+eps)^-0.5
# (mantissa-linear log2, 1.5% max err), then one Newton step
# y1 = y0*(1.5 - 0.5*v*y0^2) on Pool/DVE -> 3.3e-4 max rel err.
RSTD_SCALE = -4.1314791474e-08
RSTD_BIAS = 43.9997700144

_CACHED = {}


def _build(chain=1, post_q=False, post_k=False, do_proj=True, do_attn=True,
           gps=True, bv_zero=True, qpipe=True):
    nc = bacc.Bacc()

    xT = nc.declare_dram_parameter("xT", [C, NT], BF16, isOutput=False)
    yT = nc.declare_dram_parameter("yT", [C, NT], BF16, isOutput=False)
    wqT = nc.declare_dram_parameter("wqT", [C, DL], BF16, isOutput=False)
    wkT = nc.declare_dram_parameter("wkT", [C, DL], BF16, isOutput=False)
    wvT = nc.declare_dram_parameter("wvT", [C, NHL * VW], BF16, isOutput=False)
    wpT = nc.declare_dram_parameter("wpT", [DL, C], BF16, isOutput=False)
    # cblob cols: 0-1 bq'(dt), 2-3 bk'(dt), 4-5 invg2_q(dt), 6-7 invg2_k(dt),
    # 8 eps, 9-10 gq_post(dt), 11-12 bq_post(dt), 13-14 gk_post(dt),
    # 15-16 bk_post(dt)
    cblob = nc.declare_dram_parameter("cblob", [P, 18], F32, isOutput=False)
    # stats stationaries: [mq0, mq1, mk0, mk1, sq0, sq1, sk0, sk1]
    oselblob = nc.declare_dram_parameter("oselblob", [P, 8 * P], F32R,
                                         isOutput=False)
    rowblob = nc.declare_dram_parameter("rowblob", [1, P + NHL * VW], F32R,
                                        isOutput=False)
    ident = nc.declare_dram_parameter("ident", [P, P], F32, isOutput=False)
    out = nc.declare_dram_parameter("out", [NT, C], BF16, isOutput=True)

    from contextlib import ExitStack

    with tile.TileContext(nc) as tc:
      for _rep in range(chain):
       with ExitStack() as top:
        cp = top.enter_context(tc.tile_pool(name="const", bufs=1))
        t_cblob = cp.tile([P, 18], F32)
        t_osel = cp.tile([P, 8 * P], F32R)
        t_rowblob = cp.tile([1, P + NHL * VW], F32R)
        t_ident = cp.tile([P, P], F32)
        t_bqc = t_cblob[:, 0:2]
        t_bkc = t_cblob[:, 2:4]
        t_ig2q = t_cblob[:, 4:6]
        t_ig2k = t_cblob[:, 6:8]
        t_eps = t_cblob[:, 8:9]
        t_gqp = t_cblob[:, 9:11]
        t_bqp = t_cblob[:, 11:13]
        t_gkp = t_cblob[:, 13:15]
        t_bkp = t_cblob[:, 15:17]
        t_rstdb = t_cblob[:, 17:18]
        t_mq = [t_osel[:, 0 * P:1 * P], t_osel[:, 1 * P:2 * P]]
        t_mk = [t_osel[:, 2 * P:3 * P], t_osel[:, 3 * P:4 * P]]
        t_sq = [t_osel[:, 4 * P:5 * P], t_osel[:, 5 * P:6 * P]]
        t_sk = [t_osel[:, 6 * P:7 * P], t_osel[:, 7 * P:8 * P]]
        t_one1 = t_rowblob[:, 0:P]
        t_bvr_r = t_rowblob[:, P:P + NHL * VW]

        def _load_consts():
            nc.sync.dma_start(t_cblob[:], cblob[:])
            nc.sync.dma_start(t_osel[:], oselblob[:])
            nc.sync.dma_start(t_rowblob[:], rowblob[:])
            nc.sync.dma_start(t_ident[:], ident[:])

        pp = top.enter_context(tc.tile_pool(name="persist", bufs=1))
        kT_ln = [pp.tile([P, NT], F32R, tag=f"kTln{i}", name=f"kTln{i}")
                 for i in range(2)]
        qT_ln = [pp.tile([P, NT], F32R, tag=f"qTln{i}", name=f"qTln{i}")
                 for i in range(2)]
        v_sb = pp.tile([P, MTILES * NHL * VW], BF16, tag="v", name="v_sb")
        otn = [pp.tile([P, NT], BF16, tag=f"otn{i}", name=f"otn{i}")
               for i in range(2)]
        wp_sb = [pp.tile([P, C], BF16, tag=f"wp{i}", name=f"wp{i}")
                 for i in range(2)]

        # --- PSUM: 8 banks total ---
        stp = top.enter_context(
            tc.tile_pool(name="stp", bufs=2, space="PSUM"))      # 4 banks
        scr = top.enter_context(
            tc.tile_pool(name="scr", bufs=1, space="PSUM"))      # 2 banks
        otp = top.enter_context(
            tc.tile_pool(name="otp", bufs=1, space="PSUM"))      # 2 banks
        otA = otp.tile([P, CH], F32, tag="otA", name="otA")
        otB = otp.tile([P, CH], F32, tag="otB", name="otB")

        def scr_long():
            return scr.tile([P, CH], F32, tag="qps", name="qps", bufs=1)

        def scr_short():
            return scr.tile([P, CH], F32, tag="short", name="shrt", bufs=1)

        # --- SBUF work pools ---
        ablock = top.enter_context(tc.tile_pool(name="ablk", bufs=3))
        sc_pool = top.enter_context(tc.tile_pool(name="sc", bufs=1))
        ptp = top.enter_context(tc.tile_pool(name="pt", bufs=6))
        rcp = top.enter_context(tc.tile_pool(name="rcp", bufs=1))
        obp = top.enter_context(tc.tile_pool(name="outsb", bufs=3))

        _zt_cache = []

        def _zero_tile():
            if not _zt_cache:
                z = sc_pool.tile([P, CH], F32R, tag="zz", name="zz", bufs=1)
                nc.vector.memset(z[:], 0.0)
                _zt_cache.append(z)
            return _zt_cache[0][:]

        def _bcast_col(bp_col, dt):
            bt = sc_pool.tile([P, CH], F32R, tag="bt", name="bt", bufs=2)
            nc.gpsimd.tensor_scalar_add(bt[:], _zero_tile(),
                                        bp_col[:, dt:dt + 1])
            return bt[:]

        def make_proj_quanta(src3, wT_sb, bias_col, ig2_col, m_st, s_st,
                             post, gp_col, bp_col, lnout, do_v,
                             ch, wvT_sb=None, add_on_dve=False,
                             after_dma=None, split=False, head=False):
            """Quanta (thunks) for one chunk of a projection (K or Q).
            PE cost per quantum kept under ~450ns so attention slots can
            absorb them."""
            quanta = []
            box = {}

            def q_dma():
                yt = ablock.tile([P, 8 * CH], BF16, tag="ablock",
                                 name="ablock")
                y2 = yt[:].rearrange("p (c n) -> p c n", n=CH)
                nq = 4 if (head and ch == 0 and do_v) else 2
                stepq = 8 // nq
                for qi in range(nq):
                    nc.sync.dma_start(
                        y2[:, qi * stepq:(qi + 1) * stepq, :],
                        src3[:, qi * stepq:(qi + 1) * stepq,
                             ch * CH:(ch + 1) * CH])
                box["yt"] = yt
                if after_dma is not None:
                    after_dma()
            quanta.append(q_dma)

            def mk_mm(dt, ct0):
                def q():
                    yt = box["yt"]
                    if ("ps", dt) not in box:
                        if head:
                            if "stps" not in box:
                                box["stps"] = stp.tile([P, 2 * CH], F32,
                                                       name="st")
                            box[("ps", dt)] = box["stps"][
                                :, dt * CH:(dt + 1) * CH]
                        else:
                            box[("ps", dt)] = scr_long()
                    ps = box[("ps", dt)]
                    for ct in range(ct0, ct0 + 2):
                        nc.tensor.matmul(
                            ps[:],
                            wT_sb[:, ct * DL + dt * P: ct * DL + (dt + 1) * P],
                            yt[:, ct * CH:(ct + 1) * CH],
                            start=(ct == 0), stop=(ct == 7),
                        )
                return q

            def mk_evac(dt):
                def q():
                    ps = box[("ps", dt)]
                    raw = sc_pool.tile([P, CH], F32R, tag="raw", name="raw",
                                       bufs=6)
                    if add_on_dve:
                        nc.vector.tensor_scalar_add(
                            raw[:], ps[:], bias_col[:, dt:dt + 1])
                    else:
                        nc.scalar.add(raw[:], ps[:], bias_col[:, dt:dt + 1])
                    sq = sc_pool.tile([P, CH], F32R, tag="sq", name="sq",
                                      bufs=4)
                    if gps and not head:
                        nc.gpsimd.tensor_mul(sq[:], raw[:], raw[:])
                    else:
                        nc.vector.tensor_mul(sq[:], raw[:], raw[:])
                    box[("raw", dt)] = raw
                    box[("sq", dt)] = sq
                return q

            def mk_vq(j):
                def q():
                    yt = box["yt"]
                    vt = otA if j % 2 == 0 else otB
                    vp = vt[:, 0:NHL * VW]
                    for ct in range(8):
                        nc.tensor.matmul(
                            vp,
                            yt[:, ct * CH + j * P: ct * CH + (j + 1) * P],
                            wvT_sb[:, ct * NHL * VW:(ct + 1) * NHL * VW],
                            start=(ct == 0), stop=(ct == 7 and bv_zero),
                        )
                    if not bv_zero:
                        nc.tensor.matmul(
                            vp, t_one1[0:1, 0:P], t_bvr_r[0:1, :],
                            start=False, stop=True)
                    m = 4 * ch + j
                    if bv_zero:
                        dst = v_sb[:, m * NHL * VW:(m + 1) * NHL * VW] \
                            .rearrange("p (h w) -> p h w", w=VW)[:, :, 0:HD]
                        src = vp.rearrange("p (h w) -> p h w",
                                           w=VW)[:, :, 0:HD]
                        nc.vector.tensor_copy(dst, src)
                    else:
                        nc.vector.tensor_copy(
                            v_sb[:, m * NHL * VW:(m + 1) * NHL * VW], vp)
                return q

            def mk_smean(dt):
                def q():
                    sm = scr_short()
                    nc.tensor.matmul(sm[:], m_st[dt], box[("raw", dt)][:],
                                     start=True, stop=True)
                    t1a = sc_pool.tile([P, CH], F32, tag="t1a", name="t1a",
                                       bufs=3)
                    nc.vector.tensor_scalar_mul(t1a[:], sm[:],
                                                ig2_col[:, dt:dt + 1])
                    t1 = sc_pool.tile([P, CH], F32, tag="t1", name="t1",
                                      bufs=3)
                    nc.vector.tensor_mul(t1[:], t1a[:], sm[:])
                    diff = sc_pool.tile([P, CH], F32R, tag="diff",
                                        name="diff", bufs=3)
                    nc.vector.tensor_sub(diff[:], box[("raw", dt)][:],
                                         sm[:])
                    box[("t1", dt)] = t1
                    box[("diff", dt)] = diff
                return q

            def mk_smsq(dt):
                def q():
                    sm = scr_short()
                    nc.tensor.matmul(sm[:], s_st[dt], box[("sq", dt)][:],
                                     start=True, stop=True)
                    var = sc_pool.tile([P, CH], F32, tag="var", name="var",
                                       bufs=3)
                    # var = (msq + eps) - mean^2*ig2
                    nc.vector.scalar_tensor_tensor(
                        var[:], sm[:], t_eps[:, 0:1],
                        box[("t1", dt)][:], ALU.add, ALU.subtract)
                    box[("var", dt)] = var
                return q

            def mk_rstd(dt):
                def q():
                    var = box[("var", dt)]
                    y0 = sc_pool.tile([P, CH], F32, tag="y0", name="y0",
                                      bufs=2)
                    nc.scalar.activation(
                        y0[:], var[:].bitcast(mybir.dt.int32), AF.Exp,
                        scale=RSTD_SCALE, bias=t_rstdb[:, 0:1])
                    b = sc_pool.tile([P, CH], F32, tag="ntb", name="ntb",
                                     bufs=2)
                    if head:
                        ysq = sc_pool.tile([P, CH], F32, tag="ysq",
                                           name="ysq", bufs=2)
                        nc.scalar.square(ysq[:], y0[:])
                        nc.gpsimd.tensor_mul(b[:], var[:], ysq[:])
                    else:
                        a = sc_pool.tile([P, CH], F32, tag="nta",
                                         name="nta", bufs=2)
                        nc.gpsimd.tensor_mul(a[:], var[:], y0[:])
                        nc.gpsimd.tensor_mul(b[:], a[:], y0[:])
                    cc = sc_pool.tile([P, CH], F32, tag="ntc", name="ntc",
                                      bufs=2)
                    nc.vector.tensor_scalar(cc[:], b[:], -0.5, 1.5,
                                            ALU.mult, ALU.add)
                    rstd = sc_pool.tile([P, CH], F32, tag="rstd",
                                        name="rstd", bufs=3)
                    nc.vector.tensor_mul(rstd[:], cc[:], y0[:])
                    box[("rstd", dt)] = rstd
                return q

            def mk_apply(dt):
                def q():
                    sl = slice(ch * CH, (ch + 1) * CH)
                    rstd = box[("rstd", dt)][:]
                    diff = box[("diff", dt)]
                    if post:
                        zt = sc_pool.tile([P, CH], F32R, tag="zt",
                                          name="zt", bufs=2)
                        nc.gpsimd.tensor_mul(zt[:], diff[:], rstd)
                        nc.vector.scalar_tensor_tensor(
                            lnout[dt][:, sl], zt[:], gp_col[:, dt:dt + 1],
                            _bcast_col(bp_col, dt), ALU.mult, ALU.add)
                    elif gps:
                        nc.gpsimd.tensor_mul(lnout[dt][:, sl], diff[:],
                                             rstd)
                    else:
                        nc.vector.tensor_mul(lnout[dt][:, sl], diff[:],
                                             rstd)
                return q

            if split:
                heavy, light = [], []
                for dt in range(2):
                    for ct0 in range(0, 8, 2):
                        heavy.append(mk_mm(dt, ct0))
                    heavy.append(mk_evac(dt))
                    if do_v:
                        heavy.append(mk_vq(2 * dt))
                        heavy.append(mk_vq(2 * dt + 1))
                    light.append(mk_smean(dt))
                    light.append(mk_smsq(dt))
                    light.append(mk_rstd(dt))
                    light.append(mk_apply(dt))
                return quanta[0], heavy, light
            for dt in range(2):
                for ct0 in range(0, 8, 2):
                    quanta.append(mk_mm(dt, ct0))
                quanta.append(mk_evac(dt))
                if do_v:
                    quanta.append(mk_vq(2 * dt))
                    quanta.append(mk_vq(2 * dt + 1))
                quanta.append(mk_smean(dt))
                quanta.append(mk_smsq(dt))
                quanta.append(mk_rstd(dt))
                quanta.append(mk_apply(dt))
            return quanta

        def run_all(quanta):
            for q in quanta:
                q()

        # ---------------- K/V phase ----------------
        if do_proj:
            wpool = top.enter_context(tc.tile_pool(name="wkv", bufs=1))
            wkT_sb = wpool.tile([P, 8 * DL], BF16)
            wvT_sb = wpool.tile([P, 8 * NHL * VW], BF16)
            wqT_sb = wpool.tile([P, 8 * DL], BF16)
            if bv_zero:
                nc.vector.memset(v_sb[:], 1.0)
            wk2 = wkT_sb[:].rearrange("p (c d) -> p c d", d=DL)
            wk3 = wkT[:].rearrange("(c p) d -> p c d", p=P)
            nc.sync.dma_start(wk2[:, 0:4, :], wk3[:, 0:4, :])
            nc.sync.dma_start(wk2[:, 4:8, :], wk3[:, 4:8, :])

            def _load_rest():
                _load_consts()
                nc.sync.dma_start(
                    wvT_sb[:].rearrange("p (c d) -> p c d", d=NHL * VW),
                    wvT[:].rearrange("(c p) d -> p c d", p=P))
                nc.sync.dma_start(
                    wqT_sb[:].rearrange("p (c d) -> p c d", d=DL),
                    wqT[:].rearrange("(c p) d -> p c d", p=P))
                nc.sync.dma_start(wp_sb[0][:], wpT[0:P, :])
                nc.sync.dma_start(wp_sb[1][:], wpT[P:DL, :])

            y3 = yT[:].rearrange("(c p) n -> p c n", p=P)
            x3 = xT[:].rearrange("(c p) n -> p c n", p=P)

            # software-pipelined K chunks + Q(0): dma issued one chunk
            # ahead; chunk ch's stats chains (light) interleave with chunk
            # ch+1's matmul stream (heavy) to keep PE warm.
            parts = [make_proj_quanta(
                y3, wkT_sb, t_bkc, t_ig2k, t_mk, t_sk,
                post_k, t_gkp, t_bkp, kT_ln, True,
                ch, wvT_sb=wvT_sb,
                after_dma=_load_rest if ch == 0 else None, split=True,
                head=True)
                for ch in range(NCH)]

            def q_quanta(ch, split=False, head=False):
                # head=True: ACT is idle pre-attention, keep the bias-evac
                # there; pipelined chunks (1..3) must keep ACT exp-only.
                return make_proj_quanta(
                    x3, wqT_sb, t_bqc, t_ig2q, t_mq, t_sq,
                    post_q, t_gqp, t_bqp, qT_ln, False,
                    ch, add_on_dve=not head, split=split, head=head)

            # head order: K0 K1 K2 Q0 K3; K3's stats chain is deferred
            # into the first attention iteration's slots (S(m>=12) is the
            # first consumer of K3's LN output).
            parts = parts[0:3] + [q_quanta(0, split=True, head=True)] \
                + parts[3:4]
            parts[0][0]()                     # dma(0)
            deferred_light = []
            for ci in range(len(parts)):
                _, heavy, light = parts[ci]
                for i, q in enumerate(heavy):
                    q()
                    if i == 2 and ci + 1 < len(parts):
                        parts[ci + 1][0]()    # prefetch next chunk's dma
                if ci == len(parts) - 1:
                    deferred_light = light    # K3 stats -> attention slots
                else:
                    run_all(light)

            if not qpipe:
                for ch in range(1, NCH):
                    run_all(q_quanta(ch))
        else:
            _load_consts()
            nc.sync.dma_start(wp_sb[0][:], wpT[0:P, :])
            nc.sync.dma_start(wp_sb[1][:], wpT[P:DL, :])
            for t in (kT_ln[0], kT_ln[1], qT_ln[0], qT_ln[1], v_sb):
                nc.vector.memset(t[:], 0.125)

        # ---------------- attention ----------------
        if do_attn:
            ob_box = {}

            def emit_proj_subunit(ch, j, cc, alt=False):
                ntile = ch * 4 + j
                if cc == 0:
                    ob_box[(ch, j)] = obp.tile([P, C], BF16, tag="ob",
                                               name="ob")
                ob = ob_box[(ch, j)]
                pj = scr_long() if alt else scr_short()
                nc.tensor.matmul(
                    pj[:], otn[0][:, ntile * P:(ntile + 1) * P],
                    wp_sb[0][:, cc * CH:(cc + 1) * CH],
                    start=True, stop=False)
                nc.tensor.matmul(
                    pj[:], otn[1][:, ntile * P:(ntile + 1) * P],
                    wp_sb[1][:, cc * CH:(cc + 1) * CH],
                    start=False, stop=True)
                if alt:
                    nc.scalar.copy(ob[:, cc * CH:(cc + 1) * CH], pj[:])
                else:
                    nc.vector.tensor_copy(ob[:, cc * CH:(cc + 1) * CH],
                                          pj[:])
                if cc == 1:
                    nc.sync.dma_start(out[ntile * P:(ntile + 1) * P, :],
                                      ob[:])
                    del ob_box[(ch, j)]

            def emit_S(p, sl, m):
                st = stp.tile([P, 2 * CH], F32, name="st")
                nc.tensor.matmul(
                    st[:, 0:CH],
                    kT_ln[p][0:HD, m * P:(m + 1) * P],
                    qT_ln[p][0:HD, sl],
                    start=True, stop=True, tile_position=(0, 0))
                nc.tensor.matmul(
                    st[:, CH:2 * CH],
                    kT_ln[p][HD:P, m * P:(m + 1) * P],
                    qT_ln[p][HD:P, sl],
                    start=True, stop=True, tile_position=(64, 0))
                pt = ptp.tile([P, 2 * CH], BF16, name="pt")
                nc.scalar.activation(pt[:], st[:], AF.Exp)
                return pt

            def emit_OT(p, m, pt):
                base = m * NHL * VW
                for ns in range(4):
                    for h, ot in ((0, otA), (1, otB)):
                        nc.tensor.matmul(
                            ot[:, ns * VW:(ns + 1) * VW],
                            pt[:, h * CH + ns * P: h * CH + (ns + 1) * P],
                            v_sb[:, base + (2 * p + h) * VW:
                                 base + (2 * p + h + 1) * VW],
                            start=(m == 0 and ns == 0),
                            stop=(m == MTILES - 1 and ns == 3),
                            skip_group_check=True,
                        )

            def emit_evac1(use_act=False):
                # denominators -> reciprocal; normalize [n, d] into SBUF
                rec = rcp.tile([P, 8], F32, tag="rec", name="rec", bufs=2)
                for h, ot in ((0, otA), (1, otB)):
                    d3 = ot[:, 0:4 * VW].rearrange(
                        "p (n w) -> p n w", w=VW)[:, :, HD:VW]
                    r3 = rec[:, 4 * h:4 * h + 4].rearrange(
                        "p (n o) -> p n o", o=1)
                    nc.vector.reciprocal_approx_fast(r3, d3)
                ands = {}
                for ns in range(4):
                    for h, ot in ((0, otA), (1, otB)):
                        a = rcp.tile([P, HD], F32, tag="and", name="a_nd",
                                     bufs=10)
                        if use_act and h == 1:
                            nc.scalar.mul(
                                a[:], ot[:, ns * VW:ns * VW + HD],
                                rec[:, 4 * h + ns:4 * h + ns + 1])
                        else:
                            nc.vector.tensor_scalar_mul(
                                a[:], ot[:, ns * VW:ns * VW + HD],
                                rec[:, 4 * h + ns:4 * h + ns + 1])
                        ands[(h, ns)] = a
                return ands

            def emit_transp(p_prev, ch_prev, ns, ands, alt=False):
                for h in range(2):
                    a = ands[(h, ns)]
                    s = scr_long() if (alt and h == 1) else scr_short()
                    tp = s[0:HD, 0:P]
                    nc.tensor.matmul(tp, a[:], t_ident[:],
                                     is_transpose=True)
                    dst = otn[p_prev][h * HD:(h + 1) * HD,
                                      ch_prev * CH + ns * P:
                                      ch_prev * CH + (ns + 1) * P]
                    if alt and h == 1:
                        nc.scalar.copy(dst, tp)
                    else:
                        nc.vector.tensor_copy(dst, tp)

            pending_proj = None     # ch whose out-proj is due
            prev = None             # (p, ch, ands) awaiting transpose
            qq = list(deferred_light) if do_proj else []
            iters = [(ch, p) for ch in range(NCH) for p in range(2)]
            carry = {}
            for idx, (ch, p) in enumerate(iters):
                sl = slice(ch * CH, (ch + 1) * CH)
                if (qpipe and do_proj and p == 0 and ch < NCH - 1):
                    qq = qq + q_quanta(ch + 1)
                pts = carry
                carry = {}
                for m in range(MTILES):
                    if m not in pts:
                        pts[m] = emit_S(p, sl, m)
                    if m > 0:
                        emit_OT(p, m - 1, pts.pop(m - 1))
                    if m % 2 == 0 and m < 8 and prev is not None:
                        emit_transp(prev[0], prev[1], m // 2, prev[2])
                        if m == 6:
                            prev = None
                    if (p == 0 and pending_proj is not None
                            and m % 2 == 1):
                        emit_proj_subunit(pending_proj,
                                          (m - 1) // 4,
                                          ((m - 1) // 2) % 2)
                        if m == 15:
                            pending_proj = None
                    if qq and (p == 1 or m >= 8 or ch == 0):
                        qq.pop(0)()
                    if m == MTILES - 1 and idx + 1 < len(iters):
                        # prefetch next iteration's S(0)+exp(0) so ACT
                        # never idles across the (ch, p) boundary
                        nch, np_ = iters[idx + 1]
                        nsl = slice(nch * CH, (nch + 1) * CH)
                        carry[0] = emit_S(np_, nsl, 0)
                last = (idx == len(iters) - 1)
                emit_OT(p, MTILES - 1, pts.pop(MTILES - 1))
                ands = emit_evac1(use_act=last)
                prev = (p, ch, ands)
                if p == 1:
                    pending_proj = ch
            while qq:
                qq.pop(0)()
            # tail: interleave last transposes with last out-proj units,
            # alternating scratch banks so PE never waits a DVE evac.
            for j in range(4):
                emit_transp(prev[0], prev[1], j, prev[2], alt=True)
                emit_proj_subunit(pending_proj, j, 0)
                emit_proj_subunit(pending_proj, j, 1, alt=True)

    nc.finalize()
    return nc


def _get_nc(post_q, post_k, bv_zero):
    key = (post_q, post_k, bv_zero)
    if key not in _CACHED:
        _CACHED[key] = _build(post_q=post_q, post_k=post_k, bv_zero=bv_zero)
    return _CACHED[key]


def _host_inputs(x, y, Wq, bq, Wkv, bkv, q_gamma, q_beta, k_gamma, k_beta,
                 Wproj, bproj):
    f = np.float32
    try:
        import ml_dtypes
        bf = ml_dtypes.bfloat16
    except ImportError:  # pragma: no cover
        import jax.numpy as jnp
        bf = jnp.bfloat16

    post_q = bool(np.any(np.abs(q_gamma) < 1e-5) or np.any(q_beta != 0))
    post_k = bool(np.any(np.abs(k_gamma) < 1e-5) or np.any(k_beta != 0))
    gq = (np.ones(DL, f) * SCALE if post_q
          else np.tile(q_gamma.astype(f), NHL) * SCALE)
    gk = (np.ones(DL, f) if post_k else np.tile(k_gamma.astype(f), NHL))

    def _stats_stationaries(g):
        m_st, s_st = [], []
        for dt in range(2):
            gdt = g[dt * P:(dt + 1) * P]
            m1 = np.zeros((P, P), f)
            m2 = np.zeros((P, P), f)
            for h in range(2):
                s = slice(h * HD, (h + 1) * HD)
                gg = gdt[s]
                m1[s, s] = gg[None, :] / (HD * gg[:, None])
                m2[s, s] = 1.0 / (HD * gg[:, None] ** 2)
            m_st.append(m1)
            s_st.append(m2)
        return m_st, s_st

    mq, sq = _stats_stationaries(gq)
    mk, sk = _stats_stationaries(gk)

    in_maps = []
    for c in range(8):
        b, hg = divmod(c, 4)
        hs = hg * DL
        xT = np.ascontiguousarray(x[b].T).astype(bf)
        yT = np.ascontiguousarray(y[b].T).astype(bf)
        wqT_ = np.ascontiguousarray(
            (Wq[hs:hs + DL] * gq[:, None]).T).astype(bf)
        wkT_ = np.ascontiguousarray(
            (Wkv[hs:hs + DL] * gk[:, None]).T).astype(bf)
        Wv_s = Wkv[C + hs: C + hs + DL]
        wvT_ = np.zeros((C, NHL * VW), f)
        bvr_r = np.zeros((1, NHL * VW), f)
        bv_s = bkv[C + hs: C + hs + DL]
        for h in range(NHL):
            wvT_[:, h * VW:h * VW + HD] = Wv_s[h * HD:(h + 1) * HD].T
            bvr_r[0, h * VW:h * VW + HD] = bv_s[h * HD:(h + 1) * HD]
            bvr_r[0, h * VW + HD] = 1.0
        wvT_ = wvT_.astype(bf)
        wpT_ = np.ascontiguousarray(Wproj[:, hs:hs + DL].T).astype(bf)
        cb = np.zeros((P, 18), f)
        cb[:, 17] = RSTD_BIAS
        cb[:, 0] = bq[hs:hs + P] * gq[0:P]
        cb[:, 1] = bq[hs + P:hs + DL] * gq[P:DL]
        cb[:, 2] = bkv[hs:hs + P] * gk[0:P]
        cb[:, 3] = bkv[hs + P:hs + DL] * gk[P:DL]
        cb[:, 4] = 1.0 / gq[0:P] ** 2
        cb[:, 5] = 1.0 / gq[P:DL] ** 2
        cb[:, 6] = 1.0 / gk[0:P] ** 2
        cb[:, 7] = 1.0 / gk[P:DL] ** 2
        cb[:, 8] = LN_EPS
        if post_q:
            cb[:, 9] = np.tile(q_gamma.astype(f), 2)
            cb[:, 10] = np.tile(q_gamma.astype(f), 2)
            cb[:, 11] = np.tile(q_beta.astype(f) * SCALE, 2)
            cb[:, 12] = np.tile(q_beta.astype(f) * SCALE, 2)
        if post_k:
            cb[:, 13] = np.tile(k_gamma.astype(f), 2)
            cb[:, 14] = np.tile(k_gamma.astype(f), 2)
            cb[:, 15] = np.tile(k_beta.astype(f), 2)
            cb[:, 16] = np.tile(k_beta.astype(f), 2)
        oselb = np.concatenate(
            [mq[0], mq[1], mk[0], mk[1], sq[0], sq[1], sk[0], sk[1]],
            axis=1).astype(f)
        rowb = np.zeros((1, P + NHL * VW), f)
        rowb[0, 0:P] = 1.0
        rowb[0, P:] = bvr_r[0]
        in_maps.append({
            "xT": xT, "yT": yT, "wqT": wqT_, "wkT": wkT_, "wvT": wvT_,
            "wpT": wpT_, "cblob": cb, "oselblob": oselb,
            "rowblob": rowb, "ident": np.eye(P, dtype=f),
        })
    return in_maps, post_q, post_k


def kernel(x, y, Wq, bq, Wkv, bkv, q_gamma, q_beta, k_gamma, k_beta,
           Wproj, bproj, _trace=False, _trace_kwargs=None):
    args = [np.asarray(a, dtype=np.float32)
            for a in (x, y, Wq, bq, Wkv, bkv, q_gamma, q_beta, k_gamma,
                      k_beta, Wproj, bproj)]
    (x, y, Wq, bq, Wkv, bkv, q_gamma, q_beta, k_gamma, k_beta,
     Wproj, bproj) = args
    in_maps, post_q, post_k = _host_inputs(
        x, y, Wq, bq, Wkv, bkv, q_gamma, q_beta, k_gamma, k_beta,
        Wproj, bproj)
    bv_zero = not bool(np.any(bkv[C:]))
    nc = _get_nc(post_q, post_k, bv_zero)
    kw = {}
    if _trace:
        kw = {"trace": True, **(_trace_kwargs or {})}
    res = run_bass_kernel_spmd(nc, in_maps, list(range(8)), **kw)
    B = x.shape[0]
    out_full = np.zeros((B, NT, C), dtype=np.float32)
    for c in range(8):
        b = c // 4
        out_full[b] += np.asarray(res.results[c]["out"], dtype=np.float32)
    out_full += bproj[None, None, :]
    if _trace:
        return out_full, res
    return out_full
